# revision 16
# baseline (speedup 1.0000x reference)
"""AttnBlock2D (B=4, C=512, H=W=64) on 8 Trainium2 NeuronCores.

Strategy: data-parallel over batch x sequence-parallel over output tokens.
Core c handles image b = c//2 and output-token half h = c%2 (2048 of 4096
tokens).  Each core computes q (all tokens), k (its token half), vT (all
tokens, transposed layout) with 1x1-conv GEMMs, then attention in the
"scores-transposed" formulation S^T[j, i] = <q_j, k_i> so that the softmax
contraction axis j lands on SBUF partitions and both attention matmuls run
without any on-chip transposes:

    S^T[j, i]   = sum_c q[c, j] k[c, i]          (lhsT = q, rhs = k)
    e^T[j, i]   = exp(scale * S^T[j, i])          (ScalarE, no max-subtract:
                                                   scores ~ N(0,1), exp safe)
    s[i]        = sum_j e^T[j, i]                 (ones-column matmul)
    u[c, i]     = sum_j vT[j, c] e^T[j, i]        (lhsT = vT, rhs = e^T)
    y[co, i]    = (Wo @ u)[co, i] / s[i] + bo[co] (normalisation deferred to
                                                   the end; proj is linear in
                                                   each column i)

All matmuls use float32r (FP22 multiply, fp32 accumulate) which streams at
full PE rate for free dims >= 256.
"""

import numpy as np

import concourse.bass as bass
import concourse.tile as tile
import concourse.mybir as mybir
from concourse import bacc
from concourse.bass_utils import run_bass_kernel_spmd

B = 4
C = 512            # C_IN == C_HID
HW = 64 * 64       # tokens per image
NCORES = 8
I = HW * B // NCORES   # 2048 output tokens per core

CK = 128           # partition chunk
NB = 512           # free-dim block
NCH = C // CK      # 4
NJB = HW // CK     # 32
NIB = I // NB      # 4
NNB = HW // NB     # 8

F32 = mybir.dt.float32
F32R = mybir.dt.float32r
AF = mybir.ActivationFunctionType
SCALE = 1.0 / float(np.sqrt(float(C)))


def build_bass(reps=1):
    nc = bacc.Bacc(
        "TRN2", target_bir_lowering=False, debug=False, enable_asserts=False
    )

    x = nc.dram_tensor("x", [C, HW], F32R, kind="ExternalInput").ap()
    xi = nc.dram_tensor("xi", [C, I], F32R, kind="ExternalInput").ap()
    wqT = nc.dram_tensor("wqT", [C, C], F32R, kind="ExternalInput").ap()
    wkT = nc.dram_tensor("wkT", [C, C], F32R, kind="ExternalInput").ap()
    wvT = nc.dram_tensor("wvT", [C, C], F32R, kind="ExternalInput").ap()
    woT = nc.dram_tensor("woT", [C, C], F32R, kind="ExternalInput").ap()
    bqp = nc.dram_tensor("bqp", [CK, NCH], F32, kind="ExternalInput").ap()
    bkp = nc.dram_tensor("bkp", [CK, NCH], F32, kind="ExternalInput").ap()
    bop = nc.dram_tensor("bop", [CK, NCH], F32, kind="ExternalInput").ap()
    onesd = nc.dram_tensor("onesd", [CK, 1], F32R, kind="ExternalInput").ap()
    out = nc.dram_tensor("out", [C, I], F32, kind="ExternalOutput").ap()

    # DRAM views with the channel dim split for 128-partition DMA
    x3 = x.rearrange("(a p) n -> p a n", p=CK)      # [128, 4, 4096]
    xi3 = xi.rearrange("(a p) n -> p a n", p=CK)    # [128, 4, 2048]
    wq3d = wqT.rearrange("(a p) n -> p a n", p=CK)  # [128, 4, 512]
    wk3d = wkT.rearrange("(a p) n -> p a n", p=CK)
    wv3d = wvT.rearrange("(a p) n -> p a n", p=CK)
    wo3d = woT.rearrange("(a p) n -> p a n", p=CK)
    out3 = out.rearrange("(a p) n -> p a n", p=CK)  # [128, 4, 2048]

    with tile.TileContext(nc) as tc:
        with tc.tile_pool(name="persist", bufs=1) as persist, \
             tc.tile_pool(name="xp", bufs=2) as xp, \
             tc.tile_pool(name="ktp", bufs=1) as ktp, \
             tc.tile_pool(name="ep", bufs=2) as ep, \
             tc.tile_pool(name="aot", bufs=1) as aot, \
             tc.tile_pool(name="ftp", bufs=2) as ftp, \
             tc.tile_pool(name="accp", bufs=1) as accp, \
             tc.tile_pool(name="rp", bufs=1) as rp, \
             tc.tile_pool(name="psA", bufs=4, space="PSUM") as psA, \
             tc.tile_pool(name="psO", bufs=1, space="PSUM") as psO, \
             tc.tile_pool(name="wqv", bufs=1) as wqv:

            # ---- persistent SBUF state ----
            q = persist.tile([CK, NCH, HW], F32R, name="q")    # q[ch, n]
            vT = persist.tile([CK, NJB, C], F32R, name="vT")   # vT[j, c]
            wk3 = persist.tile([CK, NCH, C], F32R, name="wk3")
            wo3 = persist.tile([CK, NCH, C], F32R, name="wo3")
            biases = persist.tile([CK, 3 * NCH], F32, name="biases")
            ones128 = persist.tile([CK, 1], F32R, name="ones128")
            wq3 = wqv.tile([CK, NCH, C], F32R, name="wq3")
            wv3 = wqv.tile([CK, NCH, C], F32R, name="wv3")

            # weights for the first matmuls go first on the SP DMA queue;
            # everything else rides the gpsimd queue so it doesn't delay them
            for ci in range(NCH):
                nc.scalar.dma_start(out=wq3[:, ci, :], in_=wq3d[:, ci, :])
            nc.gpsimd.dma_start(out=biases[:, 0:NCH], in_=bqp)
            nc.gpsimd.dma_start(out=biases[:, NCH:2 * NCH], in_=bkp)
            nc.gpsimd.dma_start(out=biases[:, 2 * NCH:3 * NCH], in_=bop)
            nc.gpsimd.dma_start(out=ones128, in_=onesd)
            nc.gpsimd.dma_start(out=wv3, in_=wv3d)
            nc.gpsimd.dma_start(out=wk3, in_=wk3d)
            nc.gpsimd.dma_start(out=wo3, in_=wo3d)

            bq_col = lambda cc: biases[:, cc:cc + 1]
            bk_col = lambda cc: biases[:, NCH + cc:NCH + cc + 1]
            bo_col = lambda cc: biases[:, 2 * NCH + cc:2 * NCH + cc + 1]

            # ---- phase B: q (all tokens) and vT (all tokens) ----
            rep_ctx = tc.For_i(0, reps, 1) if reps > 1 else None
            if rep_ctx is not None:
                rep_ctx.__enter__()
            for nb in range(NNB):
                xt = xp.tile([CK, NCH, NB], F32R, name="xt", tag="xt")
                for ci in range(NCH):
                    nc.sync.dma_start(
                        out=xt[:, ci, :],
                        in_=x3[:, ci, nb * NB:(nb + 1) * NB],
                    )
                # q[ch, nb-block] += wqT[cin, ch].T @ x[cin, nb-block]
                for cc in range(NCH):
                    pq = psA.tile([CK, NB], F32, name="pq", tag="psA",
                                  space="PSUM")
                    for ci in range(NCH):
                        nc.tensor.matmul(
                            pq,
                            lhsT=(wq3[:, ci, cc * CK:(cc + 1) * CK]),
                            rhs=(xt[:, ci, :]),
                            start=(ci == 0), stop=(ci == NCH - 1),
                        )
                    nc.vector.tensor_scalar_add(
                        q[:, cc, nb * NB:(nb + 1) * NB], pq, bq_col(cc)
                    )
                # vT[nb-block, ch] += x[cin, j-chunk].T @ wvT[cin, ch] (+bv)
                for nn in range(NB // CK):
                    jc = nb * (NB // CK) + nn
                    pv = psA.tile([CK, C], F32, name="pv", tag="psA",
                                  space="PSUM")
                    for ci in range(NCH):
                        nc.tensor.matmul(
                            pv,
                            lhsT=(xt[:, ci, nn * CK:(nn + 1) * CK]),
                            rhs=(wv3[:, ci, :]),
                            start=(ci == 0), stop=(ci == NCH - 1),
                        )
                    nc.vector.tensor_copy(vT[:, jc, :], pv)

            # ---- phase C: k, attention, projection per 512-token i-block ----
            def k_block(ib):
                # k for i-block ib (streamed from xi)
                xt2 = xp.tile([CK, NCH, NB], F32R, name="xt2", tag="xt")
                for ci in range(NCH):
                    nc.sync.dma_start(
                        out=xt2[:, ci, :],
                        in_=xi3[:, ci, ib * NB:(ib + 1) * NB],
                    )
                kt = ktp.tile([CK, NCH, NB], F32R, name="kt", tag="kt")
                for cc in range(NCH):
                    pk = psA.tile([CK, NB], F32, name="pk", tag="psA",
                                  space="PSUM")
                    for ci in range(NCH):
                        nc.tensor.matmul(
                            pk,
                            lhsT=(wk3[:, ci, cc * CK:(cc + 1) * CK]),
                            rhs=(xt2[:, ci, :]),
                            start=(ci == 0), stop=(ci == NCH - 1),
                        )
                    nc.vector.tensor_scalar_add(kt[:, cc, :], pk, bk_col(cc))
                return kt

            kt = k_block(0)
            for ib in range(NIB):
                po = [
                    psO.tile([CK, NB], F32, name=f"po{cc}", tag=f"po{cc}",
                             space="PSUM")
                    for cc in range(NCH)
                ]
                psum = psA.tile([1, NB], F32, name="psum", tag="psA",
                                space="PSUM")
                acc = accp.tile([CK, NB], F32R, name="acc", tag="acc")
                for jc in range(NJB):
                    ps_ = psA.tile([CK, NB], F32, name="ps", tag="psA",
                                   space="PSUM")
                    for cc in range(NCH):
                        nc.tensor.matmul(
                            ps_,
                            lhsT=(q[:, cc, jc * CK:(jc + 1) * CK]),
                            rhs=(kt[:, cc, :]),
                            start=(cc == 0), stop=(cc == NCH - 1),
                        )
                    et = ep.tile([CK, NB], F32R, name="et", tag="et")
                    nc.scalar.activation(et, ps_, AF.Exp, scale=SCALE)
                    # partial softmax denominators accumulate on DVE
                    if jc == 0:
                        nc.vector.tensor_copy(acc, et)
                    else:
                        nc.vector.tensor_add(acc, acc, et)
                    # apply: u[c, i] += vT[j, c].T @ e^T[j, i]
                    for cc in range(NCH):
                        nc.tensor.matmul(
                            po[cc],
                            lhsT=(vT[:, jc, cc * CK:(cc + 1) * CK]),
                            rhs=(et),
                            start=(jc == 0), stop=(jc == NJB - 1),
                        )
                if ib + 1 < NIB:
                    next_kt = k_block(ib + 1)
                # single cross-partition reduce: s[i] = 1^T @ acc
                nc.tensor.matmul(psum, lhsT=(ones128), rhs=(acc),
                                 start=True, stop=True)
                # normalisation factors r[i] = 1 / s[i], broadcast
                r1 = rp.tile([1, NB], F32, name="r1", tag="r1")
                nc.vector.reciprocal(r1, psum)
                rb = rp.tile([CK, NB], F32, name="rb", tag="rb")
                nc.gpsimd.partition_broadcast(rb, r1)
                # drain attention outputs to SBUF
                ao = aot.tile([CK, NCH, NB], F32R, name="ao", tag="ao")
                for cc in range(NCH):
                    nc.scalar.copy(ao[:, cc, :], po[cc])
                # projection + deferred softmax normalisation + bias
                for co in range(NCH):
                    pp = psA.tile([CK, NB], F32, name="pp", tag="psA",
                                  space="PSUM")
                    for cc in range(NCH):
                        nc.tensor.matmul(
                            pp,
                            lhsT=(wo3[:, cc, co * CK:(co + 1) * CK]),
                            rhs=(ao[:, cc, :]),
                            start=(cc == 0), stop=(cc == NCH - 1),
                        )
                    ft = ftp.tile([CK, NB], F32, name="ft", tag="ft")
                    nc.vector.tensor_mul(ft, pp, rb)
                    nc.vector.tensor_scalar_add(ft, ft, bo_col(co))
                    nc.sync.dma_start(
                        out=out3[:, co, ib * NB:(ib + 1) * NB], in_=ft
                    )
                if ib + 1 < NIB:
                    kt = next_kt
            if rep_ctx is not None:
                rep_ctx.__exit__(None, None, None)

    nc.compile()
    return nc


_NC = None


def _get_nc():
    global _NC
    if _NC is None:
        _NC = build_bass()
    return _NC


def _make_in_maps(inp, Wk, bk, Wq, bq, Wv, bv, Wo, bo):
    x_all = np.ascontiguousarray(
        np.asarray(inp, dtype=np.float32).reshape(B, C, HW)
    )
    wqT = np.ascontiguousarray(np.asarray(Wq, np.float32).T)
    wkT = np.ascontiguousarray(np.asarray(Wk, np.float32).T)
    wvT = np.ascontiguousarray(np.asarray(Wv, np.float32).T)
    woT = np.ascontiguousarray(np.asarray(Wo, np.float32).T)
    # biases packed [128, 4] so column cc is the per-partition bias of channel
    # chunk cc
    bqp = np.ascontiguousarray(np.asarray(bq, np.float32).reshape(NCH, CK).T)
    bkp = np.ascontiguousarray(np.asarray(bk, np.float32).reshape(NCH, CK).T)
    bo_eff = (np.asarray(Wo, np.float32) @ np.asarray(bv, np.float32)
              + np.asarray(bo, np.float32))
    bop = np.ascontiguousarray(bo_eff.reshape(NCH, CK).T)
    onesd = np.ones((CK, 1), np.float32)

    in_maps = []
    for c in range(NCORES):
        b, h = divmod(c, NCORES // B)
        in_maps.append({
            "x": x_all[b],
            "xi": np.ascontiguousarray(x_all[b][:, h * I:(h + 1) * I]),
            "wqT": wqT, "wkT": wkT, "wvT": wvT, "woT": woT,
            "bqp": bqp, "bkp": bkp, "bop": bop,
            "onesd": onesd,
        })
    return in_maps


def run(trace=False, tmpdir=None, **inputs):
    nc = _get_nc()
    in_maps = _make_in_maps(**inputs)
    res = run_bass_kernel_spmd(
        nc, in_maps, core_ids=list(range(NCORES)), trace=trace, tmpdir=tmpdir
    )
    full = np.empty((B, C, HW), dtype=np.float32)
    for c in range(NCORES):
        b, h = divmod(c, NCORES // B)
        full[b][:, h * I:(h + 1) * I] = res.results[c]["out"]
    return full.reshape(B, C, 64, 64), res


def kernel(**inputs):
    out, _ = run(trace=False, **inputs)
    return out


# revision 17
# speedup vs baseline: 1.0006x; 1.0006x over previous
"""AttnBlock2D (B=4, C=512, H=W=64) on 8 Trainium2 NeuronCores.

Strategy: data-parallel over batch x sequence-parallel over output tokens.
Core c handles image b = c//2 and output-token half h = c%2 (2048 of 4096
tokens).  Each core computes q (all tokens), k (its token half), vT (all
tokens, transposed layout) with 1x1-conv GEMMs, then attention in the
"scores-transposed" formulation S^T[j, i] = <q_j, k_i> so that the softmax
contraction axis j lands on SBUF partitions and both attention matmuls run
without any on-chip transposes:

    S^T[j, i]   = sum_c q[c, j] k[c, i]          (lhsT = q, rhs = k)
    e^T[j, i]   = exp(scale * S^T[j, i])          (ScalarE, no max-subtract:
                                                   scores ~ N(0,1), exp safe)
    s[i]        = sum_j e^T[j, i]                 (ones-column matmul)
    u[c, i]     = sum_j vT[j, c] e^T[j, i]        (lhsT = vT, rhs = e^T)
    y[co, i]    = (Wo @ u)[co, i] / s[i] + bo[co] (normalisation deferred to
                                                   the end; proj is linear in
                                                   each column i)

All matmuls use float32r (FP22 multiply, fp32 accumulate) which streams at
full PE rate for free dims >= 256.
"""

import numpy as np

import concourse.bass as bass
import concourse.tile as tile
import concourse.mybir as mybir
from concourse import bacc
from concourse.bass_utils import run_bass_kernel_spmd

B = 4
C = 512            # C_IN == C_HID
HW = 64 * 64       # tokens per image
NCORES = 8
I = HW * B // NCORES   # 2048 output tokens per core

CK = 128           # partition chunk
NB = 512           # free-dim block
NCH = C // CK      # 4
NJB = HW // CK     # 32
NIB = I // NB      # 4
NNB = HW // NB     # 8

F32 = mybir.dt.float32
F32R = mybir.dt.float32r
AF = mybir.ActivationFunctionType
SCALE = 1.0 / float(np.sqrt(float(C)))


def build_bass(reps=1):
    nc = bacc.Bacc(
        "TRN2", target_bir_lowering=False, debug=False, enable_asserts=False
    )

    x = nc.dram_tensor("x", [C, HW], F32R, kind="ExternalInput").ap()
    xi = nc.dram_tensor("xi", [C, I], F32R, kind="ExternalInput").ap()
    wqT = nc.dram_tensor("wqT", [C, C], F32R, kind="ExternalInput").ap()
    wkT = nc.dram_tensor("wkT", [C, C], F32R, kind="ExternalInput").ap()
    wvT = nc.dram_tensor("wvT", [C, C], F32R, kind="ExternalInput").ap()
    woT = nc.dram_tensor("woT", [C, C], F32R, kind="ExternalInput").ap()
    bqp = nc.dram_tensor("bqp", [CK, NCH], F32, kind="ExternalInput").ap()
    bkp = nc.dram_tensor("bkp", [CK, NCH], F32, kind="ExternalInput").ap()
    bop = nc.dram_tensor("bop", [CK, NCH], F32, kind="ExternalInput").ap()
    onesd = nc.dram_tensor("onesd", [CK, 1], F32R, kind="ExternalInput").ap()
    out = nc.dram_tensor("out", [C, I], F32, kind="ExternalOutput").ap()

    # DRAM views with the channel dim split for 128-partition DMA
    x3 = x.rearrange("(a p) n -> p a n", p=CK)      # [128, 4, 4096]
    xi3 = xi.rearrange("(a p) n -> p a n", p=CK)    # [128, 4, 2048]
    wq3d = wqT.rearrange("(a p) n -> p a n", p=CK)  # [128, 4, 512]
    wk3d = wkT.rearrange("(a p) n -> p a n", p=CK)
    wv3d = wvT.rearrange("(a p) n -> p a n", p=CK)
    wo3d = woT.rearrange("(a p) n -> p a n", p=CK)
    out3 = out.rearrange("(a p) n -> p a n", p=CK)  # [128, 4, 2048]

    with tile.TileContext(nc) as tc:
        with tc.tile_pool(name="persist", bufs=1) as persist, \
             tc.tile_pool(name="xp", bufs=2) as xp, \
             tc.tile_pool(name="ktp", bufs=1) as ktp, \
             tc.tile_pool(name="ep", bufs=2) as ep, \
             tc.tile_pool(name="aot", bufs=1) as aot, \
             tc.tile_pool(name="ftp", bufs=2) as ftp, \
             tc.tile_pool(name="accp", bufs=1) as accp, \
             tc.tile_pool(name="rp", bufs=1) as rp, \
             tc.tile_pool(name="psA", bufs=4, space="PSUM") as psA, \
             tc.tile_pool(name="psO", bufs=1, space="PSUM") as psO, \
             tc.tile_pool(name="wqv", bufs=1) as wqv:

            # ---- persistent SBUF state ----
            q = persist.tile([CK, NCH, HW], F32R, name="q")    # q[ch, n]
            vT = persist.tile([CK, NJB, C], F32R, name="vT")   # vT[j, c]
            wk3 = persist.tile([CK, NCH, C], F32R, name="wk3")
            wo3 = persist.tile([CK, NCH, C], F32R, name="wo3")
            biases = persist.tile([CK, 3 * NCH], F32, name="biases")
            ones128 = persist.tile([CK, 1], F32R, name="ones128")
            wq3 = wqv.tile([CK, NCH, C], F32R, name="wq3")
            wv3 = wqv.tile([CK, NCH, C], F32R, name="wv3")

            # weights for the first matmuls go first on the SP DMA queue;
            # everything else rides the gpsimd queue so it doesn't delay them
            for ci in range(NCH):
                nc.scalar.dma_start(out=wq3[:, ci, :], in_=wq3d[:, ci, :])
            nc.gpsimd.dma_start(out=biases[:, 0:NCH], in_=bqp)
            nc.gpsimd.dma_start(out=biases[:, NCH:2 * NCH], in_=bkp)
            nc.gpsimd.dma_start(out=biases[:, 2 * NCH:3 * NCH], in_=bop)
            nc.gpsimd.dma_start(out=ones128, in_=onesd)
            nc.gpsimd.dma_start(out=wv3, in_=wv3d)
            nc.gpsimd.dma_start(out=wk3, in_=wk3d)
            nc.gpsimd.dma_start(out=wo3, in_=wo3d)

            bq_col = lambda cc: biases[:, cc:cc + 1]
            bk_col = lambda cc: biases[:, NCH + cc:NCH + cc + 1]
            bo_col = lambda cc: biases[:, 2 * NCH + cc:2 * NCH + cc + 1]

            # ---- phase B: q (all tokens) and vT (all tokens) ----
            rep_ctx = tc.For_i(0, reps, 1) if reps > 1 else None
            if rep_ctx is not None:
                rep_ctx.__enter__()
            for nb in range(NNB):
                xt = xp.tile([CK, NCH, NB], F32R, name="xt", tag="xt")
                for ci in range(NCH):
                    nc.sync.dma_start(
                        out=xt[:, ci, :],
                        in_=x3[:, ci, nb * NB:(nb + 1) * NB],
                    )
                # q[ch, nb-block] += wqT[cin, ch].T @ x[cin, nb-block]
                for cc in range(NCH):
                    pq = psA.tile([CK, NB], F32, name="pq", tag="psA",
                                  space="PSUM")
                    for ci in range(NCH):
                        nc.tensor.matmul(
                            pq,
                            lhsT=(wq3[:, ci, cc * CK:(cc + 1) * CK]),
                            rhs=(xt[:, ci, :]),
                            start=(ci == 0), stop=(ci == NCH - 1),
                        )
                    nc.vector.tensor_scalar_add(
                        q[:, cc, nb * NB:(nb + 1) * NB], pq, bq_col(cc)
                    )
                # vT[nb-block, ch] += x[cin, j-chunk].T @ wvT[cin, ch] (+bv)
                for nn in range(NB // CK):
                    jc = nb * (NB // CK) + nn
                    pv = psA.tile([CK, C], F32, name="pv", tag="psA",
                                  space="PSUM")
                    for ci in range(NCH):
                        nc.tensor.matmul(
                            pv,
                            lhsT=(xt[:, ci, nn * CK:(nn + 1) * CK]),
                            rhs=(wv3[:, ci, :]),
                            start=(ci == 0), stop=(ci == NCH - 1),
                        )
                    nc.vector.tensor_copy(vT[:, jc, :], pv)

            # ---- phase C: k, attention, projection per 512-token i-block ----
            def k_block(ib):
                # k for i-block ib (streamed from xi)
                xt2 = xp.tile([CK, NCH, NB], F32R, name="xt2", tag="xt")
                for ci in range(NCH):
                    nc.sync.dma_start(
                        out=xt2[:, ci, :],
                        in_=xi3[:, ci, ib * NB:(ib + 1) * NB],
                    )
                kt = ktp.tile([CK, NCH, NB], F32R, name="kt", tag="kt")
                for cc in range(NCH):
                    pk = psA.tile([CK, NB], F32, name="pk", tag="psA",
                                  space="PSUM")
                    for ci in range(NCH):
                        nc.tensor.matmul(
                            pk,
                            lhsT=(wk3[:, ci, cc * CK:(cc + 1) * CK]),
                            rhs=(xt2[:, ci, :]),
                            start=(ci == 0), stop=(ci == NCH - 1),
                        )
                    nc.vector.tensor_scalar_add(kt[:, cc, :], pk, bk_col(cc))
                return kt

            kt = k_block(0)
            for ib in range(NIB):
                po = [
                    psO.tile([CK, NB], F32, name=f"po{cc}", tag=f"po{cc}",
                             space="PSUM")
                    for cc in range(NCH)
                ]
                psum = psA.tile([1, NB], F32, name="psum", tag="psA",
                                space="PSUM")
                acc = accp.tile([CK, NB], F32R, name="acc", tag="acc")
                for jc in range(NJB):
                    ps_ = psA.tile([CK, NB], F32, name="ps", tag="psA",
                                   space="PSUM")
                    for cc in range(NCH):
                        nc.tensor.matmul(
                            ps_,
                            lhsT=(q[:, cc, jc * CK:(jc + 1) * CK]),
                            rhs=(kt[:, cc, :]),
                            start=(cc == 0), stop=(cc == NCH - 1),
                        )
                    et = ep.tile([CK, NB], F32R, name="et", tag="et")
                    nc.scalar.activation(et, ps_, AF.Exp, scale=SCALE)
                    # partial softmax denominators accumulate on DVE
                    if jc == 0:
                        nc.vector.tensor_copy(acc, et)
                    else:
                        nc.vector.tensor_add(acc, acc, et)
                    # apply: u[c, i] += vT[j, c].T @ e^T[j, i]
                    for cc in range(NCH):
                        nc.tensor.matmul(
                            po[cc],
                            lhsT=(vT[:, jc, cc * CK:(cc + 1) * CK]),
                            rhs=(et),
                            start=(jc == 0), stop=(jc == NJB - 1),
                        )
                if ib + 1 < NIB:
                    next_kt = k_block(ib + 1)
                # single cross-partition reduce: s[i] = 1^T @ acc
                nc.tensor.matmul(psum, lhsT=(ones128), rhs=(acc),
                                 start=True, stop=True)
                # normalisation factors r[i] = 1 / s[i], broadcast
                r1 = rp.tile([1, NB], F32, name="r1", tag="r1")
                nc.vector.reciprocal(r1, psum)
                rb = rp.tile([CK, NB], F32, name="rb", tag="rb")
                nc.gpsimd.partition_broadcast(rb, r1)
                # drain attention outputs to SBUF
                ao = aot.tile([CK, NCH, NB], F32R, name="ao", tag="ao")
                for cc in range(NCH):
                    if cc % 2 == 0:
                        nc.scalar.copy(ao[:, cc, :], po[cc])
                    else:
                        nc.vector.tensor_copy(ao[:, cc, :], po[cc])
                # projection + deferred softmax normalisation + bias
                for co in range(NCH):
                    pp = psA.tile([CK, NB], F32, name="pp", tag="psA",
                                  space="PSUM")
                    for cc in range(NCH):
                        nc.tensor.matmul(
                            pp,
                            lhsT=(wo3[:, cc, co * CK:(co + 1) * CK]),
                            rhs=(ao[:, cc, :]),
                            start=(cc == 0), stop=(cc == NCH - 1),
                        )
                    ft = ftp.tile([CK, NB], F32, name="ft", tag="ft")
                    nc.vector.tensor_mul(ft, pp, rb)
                    nc.vector.tensor_scalar_add(ft, ft, bo_col(co))
                    nc.sync.dma_start(
                        out=out3[:, co, ib * NB:(ib + 1) * NB], in_=ft
                    )
                if ib + 1 < NIB:
                    kt = next_kt
            if rep_ctx is not None:
                rep_ctx.__exit__(None, None, None)

    nc.compile()
    return nc


_NC = None


def _get_nc():
    global _NC
    if _NC is None:
        _NC = build_bass()
    return _NC


def _make_in_maps(inp, Wk, bk, Wq, bq, Wv, bv, Wo, bo):
    x_all = np.ascontiguousarray(
        np.asarray(inp, dtype=np.float32).reshape(B, C, HW)
    )
    wqT = np.ascontiguousarray(np.asarray(Wq, np.float32).T)
    wkT = np.ascontiguousarray(np.asarray(Wk, np.float32).T)
    wvT = np.ascontiguousarray(np.asarray(Wv, np.float32).T)
    woT = np.ascontiguousarray(np.asarray(Wo, np.float32).T)
    # biases packed [128, 4] so column cc is the per-partition bias of channel
    # chunk cc
    bqp = np.ascontiguousarray(np.asarray(bq, np.float32).reshape(NCH, CK).T)
    bkp = np.ascontiguousarray(np.asarray(bk, np.float32).reshape(NCH, CK).T)
    bo_eff = (np.asarray(Wo, np.float32) @ np.asarray(bv, np.float32)
              + np.asarray(bo, np.float32))
    bop = np.ascontiguousarray(bo_eff.reshape(NCH, CK).T)
    onesd = np.ones((CK, 1), np.float32)

    in_maps = []
    for c in range(NCORES):
        b, h = divmod(c, NCORES // B)
        in_maps.append({
            "x": x_all[b],
            "xi": np.ascontiguousarray(x_all[b][:, h * I:(h + 1) * I]),
            "wqT": wqT, "wkT": wkT, "wvT": wvT, "woT": woT,
            "bqp": bqp, "bkp": bkp, "bop": bop,
            "onesd": onesd,
        })
    return in_maps


def run(trace=False, tmpdir=None, **inputs):
    nc = _get_nc()
    in_maps = _make_in_maps(**inputs)
    res = run_bass_kernel_spmd(
        nc, in_maps, core_ids=list(range(NCORES)), trace=trace, tmpdir=tmpdir
    )
    full = np.empty((B, C, HW), dtype=np.float32)
    for c in range(NCORES):
        b, h = divmod(c, NCORES // B)
        full[b][:, h * I:(h + 1) * I] = res.results[c]["out"]
    return full.reshape(B, C, 64, 64), res


def kernel(**inputs):
    out, _ = run(trace=False, **inputs)
    return out
